# revision 19
# baseline (speedup 1.0000x reference)
"""MoDA attention Trainium2 kernel, 8-way head-parallel.

Sharding: core c owns Q heads {2c, 2c+1} and K/V head c (their GQA group),
plus that K head's depth caches. Each core computes its heads' projections,
RoPE, joint seq+depth softmax attention, and a partial output projection
(rows 2c*128:(2c+2)*128 of Wo). Host sums the 8 partial outputs.

Structure (v2):
- Projections: K pass kt-outer (DMA-paced), V pass chunk-outer producing
  vT [d,T] with 512-col matmuls then ONE SBUF->SBUF DMA transpose to the
  [t,d] layout O needs, Q passes chunk-interleaved with their ropes so the
  scope-1 -> scope-2 transition has no rope tail. cos/sin/rope scratch live
  on the right SBUF side so phase-B pools (left, over xT) never wait on the
  rope chain.
- Phase B per unit: S matmuls -> exp -> mask -> DVE pair-adds (tree L1)
  -> half-size ones-matmul for Z (PE streams jmax/2 tiles instead of jmax)
  -> O matmuls -> epilogue. Depth: dl matmul vs all-ones stationary
  (psum rows all Z_l -> exp gives the broadcast tile), wd4p compact copy
  via small DRAM roundtrip feeds the Z depth-add matmul.
- Outproj: [128,1024] psum pairs (2 banks), one big drain per pair
  alternating DVE/Act, DMA per 128-row block. Unit order c = 0,3,2,1 so
  only one chunk's outproj remains for the tail.
"""

import os
import sys

sys.path.insert(0, "/opt/trn_rl_repo")

import numpy as np
import ml_dtypes

import concourse.bass as bass
import concourse.bass_isa as bass_isa
import concourse.tile as tile
import concourse.mybir as mybir
from concourse import bacc
from concourse.bass_utils import run_bass_kernel_spmd

BF16 = mybir.dt.bfloat16
FP32 = mybir.dt.float32
NPBF16 = ml_dtypes.bfloat16

HQ, HK, HD, DM = 16, 8, 128, 2048
L = 4
GQA = HQ // HK
SCALE = float(HD) ** -0.5
N_CORES = 8
NQH = 2  # Q heads per core
TC = 512  # T chunk (free dim of most matmuls)
DK = DM // 128  # contraction tiles

CHUNK_ORDER = [0, 3, 2, 1]

_programs = {}
last_result = None


def _ts(i, n):
    return bass.ts(i, n)


def build_program(T):
    nc = bacc.Bacc(
        "TRN2",
        target_bir_lowering=False,
        debug=False,
        enable_asserts=False,
        num_devices=N_CORES,
    )

    xT = nc.dram_tensor("xT", [DM, T], BF16, kind="ExternalInput").ap()
    wq = nc.dram_tensor("wq", [128, NQH, DK, HD], BF16, kind="ExternalInput").ap()
    wk = nc.dram_tensor("wk", [128, DK, HD], BF16, kind="ExternalInput").ap()
    wv = nc.dram_tensor("wv", [128, DK, HD], BF16, kind="ExternalInput").ap()
    wo = nc.dram_tensor("wo", [128, NQH, DM], BF16, kind="ExternalInput").ap()
    cosT = nc.dram_tensor("cosT", [HD, T], BF16, kind="ExternalInput").ap()
    sinT = nc.dram_tensor("sinT", [HD, T], BF16, kind="ExternalInput").ap()
    kdT = nc.dram_tensor("kdT", [L, HD, T], BF16, kind="ExternalInput").ap()
    vdT = nc.dram_tensor("vdT", [L, HD, T], BF16, kind="ExternalInput").ap()
    mask = nc.dram_tensor("mask", [128, 128], BF16, kind="ExternalInput").ap()
    out = nc.dram_tensor("out", [T, DM], BF16, kind="ExternalOutput").ap()
    NU_ = (T // TC) * NQH
    wdd = nc.dram_tensor("wdd", [NU_, L, TC], BF16, kind="Internal").ap()

    NCH = T // TC  # 4 chunks
    NTB = T // 128  # 16 blocks
    NU = NCH * NQH  # 8 units

    with tile.TileContext(nc) as tc:
        with tc.tile_pool(name="persist", bufs=1) as cp:
            # ---- persistent SBUF ----
            wq_sb = cp.tile([128, NQH, DK, HD], BF16)
            wk_sb = cp.tile([128, DK, HD], BF16)
            wv_sb = cp.tile([128, DK, HD], BF16)
            wo_sb = cp.tile([128, NQH, DM], BF16)
            mask_sb = cp.tile([128, 128], BF16)
            ones_sb = cp.tile([128, 128], BF16)
            nc.vector.memset(ones_sb[:], 1.0)
            qT_sb = cp.tile([128, NQH, T], BF16)
            kT_sb = cp.tile([128, T], BF16)
            v_sb = cp.tile([128, NTB, HD], BF16)
            oT_sb = cp.tile([128, NQH, T], BF16)
            tacc_sb = cp.tile([128, NU, TC], BF16)  # depth contrib per unit
            wd4p_sb = [cp.tile([4, TC], BF16, name=f"wd4p{u}") for u in range(NU)]

            with tc.tile_pool(name="phO_sb", bufs=1) as po_:
                kdT_sb = po_.tile([128, L, T], BF16)
                vdT_sb = po_.tile([128, L, T], BF16)

                # ================= scope 1: projections =================
                with tc.tile_pool(name="phA_sb", bufs=1) as ap_, \
                     tc.tile_pool(name="phA_r", bufs=1, side="right") as apr_, \
                     tc.tile_pool(name="psA", bufs=8, space="PSUM") as psA, \
                     tc.tile_pool(name="sRope", bufs=2, side="right") as sR:
                    xT_sb = ap_.tile([128, DK, T], BF16)
                    vT_sb = ap_.tile([128, T], BF16)
                    cos_sb = apr_.tile([128, T], BF16)
                    sin_sb = apr_.tile([128, T], BF16)
                    wk0_sb = apr_.tile([128, HD], BF16)

                    # ---- input DMAs ----
                    # All queues serialize through one DMA resource and
                    # contiguous chunks <512B pay 2x, so: whole-tensor
                    # transfers only (4-8KB/partition elems), xT stream up
                    # front, Q/phase-B weights behind it in consumption
                    # order. wk0 is a tiny bootstrap copy so K(0) starts
                    # ~1.5us before the full wk lands.
                    nc.sync.dma_start(xT_sb[:, 0, :], xT[_ts(0, 128), :])
                    nc.scalar.dma_start(wk0_sb[:], wk[:, 0, :])
                    nc.scalar.dma_start(xT_sb[:, 1, :], xT[_ts(1, 128), :])
                    nc.scalar.dma_start(wk_sb[:], wk[:])
                    nc.sync.dma_start(wv_sb[:], wv[:])
                    for kt in range(2, DK):
                        eng = nc.sync if kt % 2 == 0 else nc.scalar
                        eng.dma_start(xT_sb[:, kt, :], xT[_ts(kt, 128), :])
                    nc.sync.dma_start(wq_sb[:, 0], wq[:, 0])
                    nc.scalar.dma_start(cos_sb[:], cosT[:])
                    nc.sync.dma_start(wq_sb[:, 1], wq[:, 1])
                    nc.scalar.dma_start(sin_sb[:], sinT[:])
                    # phase-B data LAST so the big cache transfers never cut
                    # ahead of the xT stream on the shared DMA engines
                    nc.sync.dma_start(mask_sb[:], mask[:])
                    nc.sync.dma_start(kdT_sb[:], kdT.rearrange("l d t -> d l t"))
                    nc.sync.dma_start(vdT_sb[:], vdT.rearrange("l d t -> d l t"))
                    nc.sync.dma_start(wo_sb[:], wo[:])

                    def rope_chunk(ps, dst, c):
                        # dst = ps*cos + rotate_half(ps)*sin, all [128, TC]
                        cs = cos_sb[:, _ts(c, TC)]
                        sn = sin_sb[:, _ts(c, TC)]
                        praw = sR.tile([128, TC], BF16, tag="praw")
                        nc.scalar.copy(praw[:], ps[:])  # frees psum fast
                        prot = sR.tile([128, TC], BF16, tag="prot")
                        nc.vector.tensor_copy(prot[0:64, :], praw[64:128, :])
                        nc.vector.tensor_copy(prot[64:128, :], praw[0:64, :])
                        t1 = sR.tile([128, TC], BF16, tag="t1")
                        nc.vector.tensor_mul(t1[:], praw[:], cs)
                        t2 = sR.tile([128, TC], BF16, tag="t2")
                        nc.vector.tensor_mul(t2[:], prot[:], sn)
                        nc.vector.tensor_tensor(
                            dst[0:64, :], t1[0:64, :], t2[0:64, :],
                            op=mybir.AluOpType.subtract,
                        )
                        nc.vector.tensor_add(
                            dst[64:128, :], t1[64:128, :], t2[64:128, :]
                        )

                    # passes 1+2 interleaved: K and V kt-outer together so
                    # per-tile PE work (~1.7us) matches the serial DMA rate
                    # (~1.6us/tile); V produces vT [d, T], one xbar transpose
                    # moves it to v_sb [t, d]. V lags K by one kt so the wv
                    # arrival never stalls the PE.
                    kps = [psA.tile([128, TC], FP32, tag="a", name=f"kps{c}")
                           for c in range(NCH)]
                    vps = [psA.tile([128, TC], FP32, tag="a", name=f"vps{c}")
                           for c in range(NCH)]
                    for kt in range(DK):
                        wk_t = wk0_sb[:] if kt == 0 else wk_sb[:, kt, :]
                        for c in range(NCH):
                            nc.tensor.matmul(
                                kps[c][:], wk_t,
                                xT_sb[:, kt, _ts(c, TC)],
                                start=(kt == 0), stop=(kt == DK - 1),
                            )
                        if kt >= 1:
                            for c in range(NCH):
                                nc.tensor.matmul(
                                    vps[c][:], wv_sb[:, kt - 1, :],
                                    xT_sb[:, kt - 1, _ts(c, TC)],
                                    start=(kt == 1), stop=False,
                                )
                    for c in range(NCH):
                        nc.tensor.matmul(
                            vps[c][:], wv_sb[:, DK - 1, :],
                            xT_sb[:, DK - 1, _ts(c, TC)],
                            start=False, stop=True,
                        )
                    for c in range(NCH):
                        rope_chunk(kps[c], kT_sb[:, _ts(c, TC)], c)
                        nc.scalar.copy(vT_sb[:, _ts(c, TC)], vps[c][:])
                    nc.sync.dma_start_transpose(v_sb[:], vT_sb[:])

                    # passes 3+4: Q projections, chunk-interleaved with ropes
                    # in phase-B consumption order.
                    for c in CHUNK_ORDER:
                        for h in range(NQH):
                            qp = psA.tile([128, TC], FP32, tag="a",
                                          name=f"qp{c}_{h}")
                            for kt in range(DK):
                                nc.tensor.matmul(
                                    qp[:], wq_sb[:, h, kt, :],
                                    xT_sb[:, kt, _ts(c, TC)],
                                    start=(kt == 0), stop=(kt == DK - 1),
                                )
                            rope_chunk(qp, qT_sb[:, h, _ts(c, TC)], c)

                # ============ scope 2: depth weights + attention ============
                with tc.tile_pool(name="psS", bufs=3, space="PSUM") as psS, \
                     tc.tile_pool(name="psO", bufs=2, space="PSUM") as psO, \
                     tc.tile_pool(name="psZ", bufs=1, space="PSUM") as psZ, \
                     tc.tile_pool(name="psOut", bufs=2, space="PSUM") as psOut, \
                     tc.tile_pool(name="sEu", bufs=3) as sEu, \
                     tc.tile_pool(name="sDb", bufs=3) as sDb, \
                     tc.tile_pool(name="sBc", bufs=4) as sBc, \
                     tc.tile_pool(name="sTt", bufs=1) as sTt, \
                     tc.tile_pool(name="sU", bufs=34) as sU, \
                     tc.tile_pool(name="sU2", bufs=18) as sU2, \
                     tc.tile_pool(name="sZb", bufs=2, side="right") as sZb, \
                     tc.tile_pool(name="sOs", bufs=2, side="right") as sOs, \
                     tc.tile_pool(name="sRes", bufs=4, side="right") as sRes:

                    units = [(c, h) for c in CHUNK_ORDER for h in range(NQH)]

                    def depth_front(u):
                        # dl = sum_d q*kd via gpsimd partition_all_reduce
                        # (broadcast f32 out), exp gives the broadcast wd_l
                        # tile; keeps the dl reduction off the PE.
                        c, h = units[u]
                        bcs = sBc.tile([128, L, TC], BF16, tag="bcs")
                        for l in range(L):
                            eu = sEu.tile([128, TC], BF16, tag="eu")
                            nc.vector.tensor_mul(
                                eu[:], qT_sb[:, h, _ts(c, TC)],
                                kdT_sb[:, l, _ts(c, TC)],
                            )
                            dlb = sDb.tile([128, TC], FP32, tag="dlb")
                            nc.gpsimd.partition_all_reduce(
                                dlb[:], eu[:], channels=128,
                                reduce_op=bass_isa.ReduceOp.add,
                            )
                            nc.scalar.activation(
                                bcs[:, l, :], dlb[:],
                                mybir.ActivationFunctionType.Exp, scale=SCALE,
                            )
                        nc.gpsimd.dma_start(wdd[u], bcs[0:1, :, :])
                        nc.gpsimd.dma_start(wd4p_sb[u][:], wdd[u])
                        return bcs

                    def depth_tacc(u, bcs):
                        # tacc = sum_l vdT_l * wd_l  (batched DVE)
                        c, h = units[u]
                        tmp4 = sTt.tile([128, L, TC], BF16, tag="tmp4")
                        nc.vector.tensor_mul(
                            tmp4[:], vdT_sb[:, :, _ts(c, TC)], bcs[:]
                        )
                        ta2 = sTt.tile([128, TC], BF16, tag="ta2")
                        nc.vector.tensor_add(ta2[:], tmp4[:, 0, :], tmp4[:, 1, :])
                        ta3 = sTt.tile([128, TC], BF16, tag="ta3")
                        nc.vector.tensor_add(ta3[:], tmp4[:, 2, :], tmp4[:, 3, :])
                        nc.vector.tensor_add(tacc_sb[:, u, :], ta2[:], ta3[:])

                    def s_phase(k):
                        c, h = units[k]
                        jmax = (c + 1) * 4
                        c0 = c * 4
                        us = []
                        for jb in range(jmax):
                            off = max(0, jb - c0) * 128
                            sp = psS.tile([128, TC], FP32, tag="s")
                            nc.tensor.matmul(
                                sp[:, off:TC], kT_sb[:, _ts(jb, 128)],
                                qT_sb[:, h, c * TC + off:(c + 1) * TC],
                                start=True, stop=True,
                            )
                            uu = sU.tile([128, TC], BF16, tag="u")
                            nc.scalar.activation(
                                uu[:, off:TC], sp[:, off:TC],
                                mybir.ActivationFunctionType.Exp, scale=SCALE,
                            )
                            if jb >= c0:
                                nc.gpsimd.tensor_mul(
                                    uu[:, off:off + 128], uu[:, off:off + 128],
                                    mask_sb[:],
                                )
                            us.append((jb, off, uu))
                        # tree level 1: DVE pair-adds halve what Z streams
                        zparts = []
                        for m in range(jmax // 2):
                            (_, oa, ua), (_, ob, ub) = us[2 * m], us[2 * m + 1]
                            u2 = sU2.tile([128, TC], BF16, tag="u2")
                            if oa < ob:
                                nc.vector.tensor_copy(
                                    u2[:, oa:ob], ua[:, oa:ob]
                                )
                            nc.vector.tensor_add(
                                u2[:, ob:TC], ua[:, ob:TC], ub[:, ob:TC]
                            )
                            zparts.append((oa, u2))
                        return us, zparts

                    def z_phase(k, zparts):
                        zp = psZ.tile([128, TC], FP32, tag="z")
                        for i, (oa, u2) in enumerate(zparts):
                            nc.tensor.matmul(
                                zp[:, oa:TC], ones_sb[:], u2[:, oa:TC],
                                start=(i == 0), stop=False,
                            )
                        nc.tensor.matmul(
                            zp[:], ones_sb[0:4, :], wd4p_sb[k][:],
                            start=False, stop=True,
                        )
                        zb = sZb.tile([128, TC], FP32, tag="zb")
                        nc.vector.reciprocal_approx_fast(zb[:], zp[:])
                        return zb

                    def o_phase(k, us):
                        op = psO.tile([128, TC], FP32, tag="o")
                        for jb, off, uu in us:
                            nc.tensor.matmul(
                                op[:, off:TC], v_sb[:, jb, :], uu[:, off:TC],
                                start=(jb == 0), stop=(jb == len(us) - 1),
                            )
                        return op

                    def epilogue(k, op, zb):
                        c, h = units[k]
                        osum = sOs.tile([128, TC], FP32, tag="osum")
                        nc.vector.tensor_add(osum[:], op[:], tacc_sb[:, k, :])
                        nc.vector.tensor_mul(
                            oT_sb[:, h, _ts(c, TC)], osum[:], zb[:]
                        )

                    ndrain = [0]
                    res_map = {}

                    def outproj_half(tb, half):
                        # 2 nch, each: 2 head matmuls into a [128,512] psum
                        # single, drain alternating DVE/Act, DMA per block.
                        if half == 0:
                            res_map[tb] = sRes.tile([128, DM], BF16, tag="res",
                                                    name=f"res{tb}")
                        res = res_map[tb]
                        for s_ in range(2):
                            nch = half * 2 + s_
                            opp = psOut.tile([128, TC], FP32, tag="op",
                                             name=f"op{tb}_{nch}")
                            for h in range(NQH):
                                nc.tensor.matmul(
                                    opp[:], oT_sb[:, h, _ts(tb, 128)],
                                    wo_sb[:, h, _ts(nch, TC)],
                                    start=(h == 0), stop=(h == NQH - 1),
                                )
                            ndrain[0] += 1
                            dst = res[:, _ts(nch, TC)]
                            if ndrain[0] % 2 == 0:
                                nc.scalar.copy(dst, opp[:])
                            else:
                                nc.vector.tensor_copy(dst, opp[:])
                        if half == 1:
                            nc.sync.dma_start(out[_ts(tb, 128), :], res[:])
                            del res_map[tb]

                    def outproj_tail(tb, half, psum_src, split=False):
                        # tail variant: per-512 singles cycling the by-now
                        # idle psS/psO/psZ banks so the drain pipeline is
                        # wider than the lone psOut pair.
                        if half == 0:
                            res_map[tb] = sRes.tile([128, DM], BF16, tag="res", name=f"res{tb}")
                        res = res_map[tb]
                        for s_ in range(2):
                            nch = half * 2 + s_
                            pool, tag = psum_src[(2 * tb + half + s_) % len(psum_src)]
                            opp = pool.tile([128, TC], FP32, tag=tag,
                                            name=f"ot{tb}_{nch}")
                            for h in range(NQH):
                                nc.tensor.matmul(
                                    opp[:], oT_sb[:, h, _ts(tb, 128)],
                                    wo_sb[:, h, _ts(nch, TC)],
                                    start=(h == 0), stop=(h == NQH - 1),
                                )
                            ndrain[0] += 1
                            dst = res[:, _ts(nch, TC)]
                            if ndrain[0] % 2 == 0:
                                nc.scalar.copy(dst, opp[:])
                            else:
                                nc.vector.tensor_copy(dst, opp[:])
                        if split:
                            nc.sync.dma_start(
                                out[_ts(tb, 128),
                                    half * (DM // 2):(half + 1) * (DM // 2)],
                                res[:, half * (DM // 2):(half + 1) * (DM // 2)],
                            )
                            if half == 1:
                                del res_map[tb]
                        elif half == 1:
                            nc.sync.dma_start(out[_ts(tb, 128), :], res[:])
                            del res_map[tb]

                    pend_out = []

                    def drain_out(n):
                        while n > 0 and pend_out:
                            outproj_half(*pend_out.pop(0))
                            n -= 1

                    # --- interleaved emission: hoisted S units fill the PE
                    # while ropes/eu drain on DVE; depth fronts stay ahead
                    # of their z-phase use; taccs lag into phase B ---
                    saved = {}
                    fronts = {}
                    saved[0] = (s_phase(0),)
                    fronts[0] = depth_front(0)
                    saved[1] = (s_phase(1),)
                    fronts[1] = depth_front(1)
                    saved[2] = (s_phase(2),)
                    fronts[2] = depth_front(2)

                    for k in range(NU):
                        if k not in saved:
                            saved[k] = (s_phase(k),)
                        nf = k + 3
                        if nf < NU and nf not in fronts:
                            fronts[nf] = depth_front(nf)
                        if k in fronts:
                            depth_tacc(k, fronts.pop(k))
                        drain_out(2)
                        if k > 0:
                            us_prev, zparts_prev = saved[k - 1][0]
                            zb = z_phase(k - 1, zparts_prev)
                            drain_out(2)
                            op = o_phase(k - 1, us_prev)
                            saved[k - 1] = (op, zb)
                        if k > 1:
                            op, zb = saved.pop(k - 2)
                            epilogue(k - 2, op, zb)
                            c2, h2 = units[k - 2]
                            if h2 == NQH - 1:
                                pend_out.extend(
                                    (tb, half)
                                    for tb in range(c2 * 4, (c2 + 1) * 4)
                                    for half in range(2)
                                )
                            if k >= 6:
                                drain_out(2)

                    us7, zparts7 = saved[7][0]
                    zb7 = z_phase(7, zparts7)
                    drain_out(2)
                    op7 = o_phase(7, us7)
                    op6, zb6 = saved.pop(6)
                    epilogue(6, op6, zb6)
                    drain_out(2)
                    epilogue(7, op7, zb7)
                    c7, _ = units[7]
                    pend_out.extend(
                        (tb, half)
                        for tb in range(c7 * 4, (c7 + 1) * 4)
                        for half in range(2)
                    )
                    # tail: cycle psS/psO/psZ banks (idle by now) alongside
                    # psOut so drains pipeline across many banks
                    psum_src = [(psS, "s"), (psO, "o"), (psZ, "z"), (psOut, "op")]
                    while pend_out:
                        tb, half = pend_out.pop(0)
                        outproj_tail(tb, half, psum_src,
                                     split=len(pend_out) < 4)

    nc.compile()
    return nc


def get_program(T):
    if T not in _programs:
        _programs[T] = build_program(T)
    return _programs[T]


def make_in_maps(x, depth_k, depth_v, cos, sin, Wq, Wk, Wv, Wo, T):
    xT16 = np.ascontiguousarray(x[0].T).astype(NPBF16)
    cosT16 = np.ascontiguousarray(cos[0, 0].T).astype(NPBF16)
    sinT16 = np.ascontiguousarray(sin[0, 0].T).astype(NPBF16)
    mask16 = np.triu(np.ones((128, 128), np.float32)).astype(NPBF16)
    in_maps = []
    for c in range(N_CORES):
        wq_c = np.ascontiguousarray(
            np.stack(
                [
                    Wq[:, (2 * c + h) * HD: (2 * c + h + 1) * HD]
                    .reshape(DK, 128, HD).transpose(1, 0, 2)
                    for h in range(NQH)
                ],
                axis=1,
            )
        ).astype(NPBF16)
        wk_c = np.ascontiguousarray(
            Wk[:, c * HD: (c + 1) * HD].reshape(DK, 128, HD).transpose(1, 0, 2)
        ).astype(NPBF16)
        wv_c = np.ascontiguousarray(
            Wv[:, c * HD: (c + 1) * HD].reshape(DK, 128, HD).transpose(1, 0, 2)
        ).astype(NPBF16)
        wo_c = np.ascontiguousarray(
            Wo[2 * c * HD: (2 * c + 2) * HD, :].reshape(NQH, HD, DM)
            .transpose(1, 0, 2)
        ).astype(NPBF16)
        kdT_c = np.ascontiguousarray(depth_k[:, 0, c].transpose(0, 2, 1)).astype(NPBF16)
        vdT_c = np.ascontiguousarray(depth_v[:, 0, c].transpose(0, 2, 1)).astype(NPBF16)
        in_maps.append(
            {
                "xT": xT16, "wq": wq_c, "wk": wk_c, "wv": wv_c, "wo": wo_c,
                "cosT": cosT16, "sinT": sinT16, "kdT": kdT_c, "vdT": vdT_c,
                "mask": mask16,
            }
        )
    return in_maps


def kernel(x, depth_k, depth_v, cos, sin, Wq, Wk, Wv, Wo):
    x = np.asarray(x, np.float32)
    T = x.shape[1]
    nc = get_program(T)
    in_maps = make_in_maps(
        x, np.asarray(depth_k, np.float32), np.asarray(depth_v, np.float32),
        np.asarray(cos, np.float32), np.asarray(sin, np.float32),
        np.asarray(Wq, np.float32), np.asarray(Wk, np.float32),
        np.asarray(Wv, np.float32), np.asarray(Wo, np.float32), T,
    )
    trace = bool(os.environ.get("MODA_TRACE"))
    res = run_bass_kernel_spmd(nc, in_maps, list(range(N_CORES)), trace=trace)
    global last_result
    last_result = res
    total = np.zeros((T, DM), np.float32)
    for c in range(N_CORES):
        total += res.results[c]["out"].astype(np.float32)
    return total.reshape(1, T, DM)


# revision 20
# speedup vs baseline: 1.0099x; 1.0099x over previous
"""MoDA attention Trainium2 kernel, 8-way head-parallel.

Sharding: core c owns Q heads {2c, 2c+1} and K/V head c (their GQA group),
plus that K head's depth caches. Each core computes its heads' projections,
RoPE, joint seq+depth softmax attention, and a partial output projection
(rows 2c*128:(2c+2)*128 of Wo). Host sums the 8 partial outputs.

Structure (v2):
- Projections: K pass kt-outer (DMA-paced), V pass chunk-outer producing
  vT [d,T] with 512-col matmuls then ONE SBUF->SBUF DMA transpose to the
  [t,d] layout O needs, Q passes chunk-interleaved with their ropes so the
  scope-1 -> scope-2 transition has no rope tail. cos/sin/rope scratch live
  on the right SBUF side so phase-B pools (left, over xT) never wait on the
  rope chain.
- Phase B per unit: S matmuls -> exp -> mask -> DVE pair-adds (tree L1)
  -> half-size ones-matmul for Z (PE streams jmax/2 tiles instead of jmax)
  -> O matmuls -> epilogue. Depth: dl matmul vs all-ones stationary
  (psum rows all Z_l -> exp gives the broadcast tile), wd4p compact copy
  via small DRAM roundtrip feeds the Z depth-add matmul.
- Outproj: [128,1024] psum pairs (2 banks), one big drain per pair
  alternating DVE/Act, DMA per 128-row block. Unit order c = 0,3,2,1 so
  only one chunk's outproj remains for the tail.
"""

import os
import sys

sys.path.insert(0, "/opt/trn_rl_repo")

import numpy as np
import ml_dtypes

import concourse.bass as bass
import concourse.bass_isa as bass_isa
import concourse.tile as tile
import concourse.mybir as mybir
from concourse import bacc
from concourse.bass_utils import run_bass_kernel_spmd

BF16 = mybir.dt.bfloat16
FP32 = mybir.dt.float32
NPBF16 = ml_dtypes.bfloat16

HQ, HK, HD, DM = 16, 8, 128, 2048
L = 4
GQA = HQ // HK
SCALE = float(HD) ** -0.5
N_CORES = 8
NQH = 2  # Q heads per core
TC = 512  # T chunk (free dim of most matmuls)
DK = DM // 128  # contraction tiles

CHUNK_ORDER = [0, 3, 2, 1]

_programs = {}
last_result = None


def _ts(i, n):
    return bass.ts(i, n)


def build_program(T):
    nc = bacc.Bacc(
        "TRN2",
        target_bir_lowering=False,
        debug=False,
        enable_asserts=False,
        num_devices=N_CORES,
    )

    xT = nc.dram_tensor("xT", [DM, T], BF16, kind="ExternalInput").ap()
    wq = nc.dram_tensor("wq", [128, NQH, DK, HD], BF16, kind="ExternalInput").ap()
    wk = nc.dram_tensor("wk", [128, DK, HD], BF16, kind="ExternalInput").ap()
    wv = nc.dram_tensor("wv", [128, DK, HD], BF16, kind="ExternalInput").ap()
    wo = nc.dram_tensor("wo", [128, NQH, DM], BF16, kind="ExternalInput").ap()
    cosT = nc.dram_tensor("cosT", [HD, T], BF16, kind="ExternalInput").ap()
    sinT = nc.dram_tensor("sinT", [HD, T], BF16, kind="ExternalInput").ap()
    kdT = nc.dram_tensor("kdT", [L, HD, T], BF16, kind="ExternalInput").ap()
    vdT = nc.dram_tensor("vdT", [L, HD, T], BF16, kind="ExternalInput").ap()
    mask = nc.dram_tensor("mask", [128, 128], BF16, kind="ExternalInput").ap()
    out = nc.dram_tensor("out", [T, DM], BF16, kind="ExternalOutput").ap()
    NU_ = (T // TC) * NQH
    wdd = nc.dram_tensor("wdd", [NU_, L, TC], BF16, kind="Internal").ap()

    NCH = T // TC  # 4 chunks
    NTB = T // 128  # 16 blocks
    NU = NCH * NQH  # 8 units

    with tile.TileContext(nc) as tc:
        with tc.tile_pool(name="persist", bufs=1) as cp:
            # ---- persistent SBUF ----
            wq_sb = cp.tile([128, NQH, DK, HD], BF16)
            wk_sb = cp.tile([128, DK, HD], BF16)
            wv_sb = cp.tile([128, DK, HD], BF16)
            wo_sb = cp.tile([128, NQH, DM], BF16)
            mask_sb = cp.tile([128, 128], BF16)
            ones_sb = cp.tile([128, 128], BF16)
            nc.vector.memset(ones_sb[:], 1.0)
            qT_sb = cp.tile([128, NQH, T], BF16)
            kT_sb = cp.tile([128, T], BF16)
            v_sb = cp.tile([128, NTB, HD], BF16)
            oT_sb = cp.tile([128, NQH, T], BF16)
            tacc_sb = cp.tile([128, NU, TC], BF16)  # depth contrib per unit
            wd4p_sb = [cp.tile([4, TC], BF16, name=f"wd4p{u}") for u in range(NU)]

            with tc.tile_pool(name="phO_sb", bufs=1) as po_:
                kdT_sb = po_.tile([128, L, T], BF16)
                vdT_sb = po_.tile([128, L, T], BF16)

                # ================= scope 1: projections =================
                with tc.tile_pool(name="phA_sb", bufs=1) as ap_, \
                     tc.tile_pool(name="phA_r", bufs=1, side="right") as apr_, \
                     tc.tile_pool(name="psA", bufs=8, space="PSUM") as psA, \
                     tc.tile_pool(name="sRope", bufs=2, side="right") as sR:
                    xT_sb = ap_.tile([128, DK, T], BF16)
                    vT_sb = ap_.tile([128, T], BF16)
                    cos_sb = apr_.tile([128, T], BF16)
                    sin_sb = apr_.tile([128, T], BF16)
                    wk0_sb = apr_.tile([128, HD], BF16)

                    # ---- input DMAs ----
                    # All queues serialize through one DMA resource and
                    # contiguous chunks <512B pay 2x, so: whole-tensor
                    # transfers only (4-8KB/partition elems), xT stream up
                    # front, Q/phase-B weights behind it in consumption
                    # order. wk0 is a tiny bootstrap copy so K(0) starts
                    # ~1.5us before the full wk lands.
                    nc.sync.dma_start(xT_sb[:, 0, :], xT[_ts(0, 128), :])
                    nc.scalar.dma_start(wk0_sb[:], wk[:, 0, :])
                    nc.scalar.dma_start(xT_sb[:, 1, :], xT[_ts(1, 128), :])
                    nc.scalar.dma_start(wk_sb[:], wk[:])
                    nc.sync.dma_start(xT_sb[:, 2, :], xT[_ts(2, 128), :])
                    nc.sync.dma_start(wv_sb[:], wv[:])
                    for kt in range(3, DK):
                        eng = nc.sync if kt % 2 == 0 else nc.scalar
                        eng.dma_start(xT_sb[:, kt, :], xT[_ts(kt, 128), :])
                    nc.sync.dma_start(wq_sb[:, 0], wq[:, 0])
                    nc.scalar.dma_start(cos_sb[:], cosT[:])
                    nc.sync.dma_start(wq_sb[:, 1], wq[:, 1])
                    nc.scalar.dma_start(sin_sb[:], sinT[:])
                    # phase-B data LAST so the big cache transfers never cut
                    # ahead of the xT stream on the shared DMA engines
                    nc.sync.dma_start(mask_sb[:], mask[:])
                    nc.sync.dma_start(kdT_sb[:], kdT.rearrange("l d t -> d l t"))
                    nc.sync.dma_start(vdT_sb[:], vdT.rearrange("l d t -> d l t"))
                    nc.sync.dma_start(wo_sb[:], wo[:])

                    def rope_chunk(ps, dst, c):
                        # dst = ps*cos + rotate_half(ps)*sin, all [128, TC]
                        cs = cos_sb[:, _ts(c, TC)]
                        sn = sin_sb[:, _ts(c, TC)]
                        praw = sR.tile([128, TC], BF16, tag="praw")
                        nc.scalar.copy(praw[:], ps[:])  # frees psum fast
                        prot = sR.tile([128, TC], BF16, tag="prot")
                        nc.vector.tensor_copy(prot[0:64, :], praw[64:128, :])
                        nc.vector.tensor_copy(prot[64:128, :], praw[0:64, :])
                        t1 = sR.tile([128, TC], BF16, tag="t1")
                        nc.vector.tensor_mul(t1[:], praw[:], cs)
                        t2 = sR.tile([128, TC], BF16, tag="t2")
                        nc.vector.tensor_mul(t2[:], prot[:], sn)
                        nc.vector.tensor_tensor(
                            dst[0:64, :], t1[0:64, :], t2[0:64, :],
                            op=mybir.AluOpType.subtract,
                        )
                        nc.vector.tensor_add(
                            dst[64:128, :], t1[64:128, :], t2[64:128, :]
                        )

                    # passes 1+2 interleaved: K and V kt-outer together so
                    # per-tile PE work (~1.7us) matches the serial DMA rate
                    # (~1.6us/tile); V produces vT [d, T], one xbar transpose
                    # moves it to v_sb [t, d]. V lags K by one kt so the wv
                    # arrival never stalls the PE.
                    kps = [psA.tile([128, TC], FP32, tag="a", name=f"kps{c}")
                           for c in range(NCH)]
                    vps = [psA.tile([128, TC], FP32, tag="a", name=f"vps{c}")
                           for c in range(NCH)]
                    for kt in range(DK):
                        wk_t = wk0_sb[:] if kt == 0 else wk_sb[:, kt, :]
                        for c in range(NCH):
                            nc.tensor.matmul(
                                kps[c][:], wk_t,
                                xT_sb[:, kt, _ts(c, TC)],
                                start=(kt == 0), stop=(kt == DK - 1),
                            )
                        if kt >= 1:
                            for c in range(NCH):
                                nc.tensor.matmul(
                                    vps[c][:], wv_sb[:, kt - 1, :],
                                    xT_sb[:, kt - 1, _ts(c, TC)],
                                    start=(kt == 1), stop=False,
                                )
                    for c in range(NCH):
                        nc.tensor.matmul(
                            vps[c][:], wv_sb[:, DK - 1, :],
                            xT_sb[:, DK - 1, _ts(c, TC)],
                            start=False, stop=True,
                        )
                    for c in range(NCH):
                        rope_chunk(kps[c], kT_sb[:, _ts(c, TC)], c)
                        nc.scalar.copy(vT_sb[:, _ts(c, TC)], vps[c][:])
                    nc.sync.dma_start_transpose(v_sb[:], vT_sb[:])

                    # passes 3+4: Q projections, chunk-interleaved with ropes
                    # in phase-B consumption order.
                    for c in CHUNK_ORDER:
                        for h in range(NQH):
                            qp = psA.tile([128, TC], FP32, tag="a",
                                          name=f"qp{c}_{h}")
                            for kt in range(DK):
                                nc.tensor.matmul(
                                    qp[:], wq_sb[:, h, kt, :],
                                    xT_sb[:, kt, _ts(c, TC)],
                                    start=(kt == 0), stop=(kt == DK - 1),
                                )
                            rope_chunk(qp, qT_sb[:, h, _ts(c, TC)], c)

                # ============ scope 2: depth weights + attention ============
                with tc.tile_pool(name="psS", bufs=3, space="PSUM") as psS, \
                     tc.tile_pool(name="psO", bufs=2, space="PSUM") as psO, \
                     tc.tile_pool(name="psZ", bufs=1, space="PSUM") as psZ, \
                     tc.tile_pool(name="psOut", bufs=2, space="PSUM") as psOut, \
                     tc.tile_pool(name="sEu", bufs=3) as sEu, \
                     tc.tile_pool(name="sDb", bufs=3) as sDb, \
                     tc.tile_pool(name="sBc", bufs=4) as sBc, \
                     tc.tile_pool(name="sTt", bufs=1) as sTt, \
                     tc.tile_pool(name="sU", bufs=34) as sU, \
                     tc.tile_pool(name="sU2", bufs=18) as sU2, \
                     tc.tile_pool(name="sZb", bufs=2, side="right") as sZb, \
                     tc.tile_pool(name="sOs", bufs=2, side="right") as sOs, \
                     tc.tile_pool(name="sRes", bufs=4, side="right") as sRes:

                    units = [(c, h) for c in CHUNK_ORDER for h in range(NQH)]

                    def depth_front(u):
                        # dl = sum_d q*kd via gpsimd partition_all_reduce
                        # (broadcast f32 out), exp gives the broadcast wd_l
                        # tile; keeps the dl reduction off the PE.
                        c, h = units[u]
                        bcs = sBc.tile([128, L, TC], BF16, tag="bcs")
                        for l in range(L):
                            eu = sEu.tile([128, TC], BF16, tag="eu")
                            nc.vector.tensor_mul(
                                eu[:], qT_sb[:, h, _ts(c, TC)],
                                kdT_sb[:, l, _ts(c, TC)],
                            )
                            dlb = sDb.tile([128, TC], FP32, tag="dlb")
                            nc.gpsimd.partition_all_reduce(
                                dlb[:], eu[:], channels=128,
                                reduce_op=bass_isa.ReduceOp.add,
                            )
                            nc.scalar.activation(
                                bcs[:, l, :], dlb[:],
                                mybir.ActivationFunctionType.Exp, scale=SCALE,
                            )
                        nc.gpsimd.dma_start(wdd[u], bcs[0:1, :, :])
                        nc.gpsimd.dma_start(wd4p_sb[u][:], wdd[u])
                        return bcs

                    def depth_tacc(u, bcs):
                        # tacc = sum_l vdT_l * wd_l  (batched DVE)
                        c, h = units[u]
                        tmp4 = sTt.tile([128, L, TC], BF16, tag="tmp4")
                        nc.vector.tensor_mul(
                            tmp4[:], vdT_sb[:, :, _ts(c, TC)], bcs[:]
                        )
                        ta2 = sTt.tile([128, TC], BF16, tag="ta2")
                        nc.vector.tensor_add(ta2[:], tmp4[:, 0, :], tmp4[:, 1, :])
                        ta3 = sTt.tile([128, TC], BF16, tag="ta3")
                        nc.vector.tensor_add(ta3[:], tmp4[:, 2, :], tmp4[:, 3, :])
                        nc.vector.tensor_add(tacc_sb[:, u, :], ta2[:], ta3[:])

                    def s_phase(k):
                        c, h = units[k]
                        jmax = (c + 1) * 4
                        c0 = c * 4
                        us = []
                        for jb in range(jmax):
                            off = max(0, jb - c0) * 128
                            sp = psS.tile([128, TC], FP32, tag="s")
                            nc.tensor.matmul(
                                sp[:, off:TC], kT_sb[:, _ts(jb, 128)],
                                qT_sb[:, h, c * TC + off:(c + 1) * TC],
                                start=True, stop=True,
                            )
                            uu = sU.tile([128, TC], BF16, tag="u")
                            nc.scalar.activation(
                                uu[:, off:TC], sp[:, off:TC],
                                mybir.ActivationFunctionType.Exp, scale=SCALE,
                            )
                            if jb >= c0:
                                nc.gpsimd.tensor_mul(
                                    uu[:, off:off + 128], uu[:, off:off + 128],
                                    mask_sb[:],
                                )
                            us.append((jb, off, uu))
                        # tree level 1: DVE pair-adds halve what Z streams
                        zparts = []
                        for m in range(jmax // 2):
                            (_, oa, ua), (_, ob, ub) = us[2 * m], us[2 * m + 1]
                            u2 = sU2.tile([128, TC], BF16, tag="u2")
                            if oa < ob:
                                nc.vector.tensor_copy(
                                    u2[:, oa:ob], ua[:, oa:ob]
                                )
                            nc.vector.tensor_add(
                                u2[:, ob:TC], ua[:, ob:TC], ub[:, ob:TC]
                            )
                            zparts.append((oa, u2))
                        return us, zparts

                    def z_phase(k, zparts):
                        zp = psZ.tile([128, TC], FP32, tag="z")
                        for i, (oa, u2) in enumerate(zparts):
                            nc.tensor.matmul(
                                zp[:, oa:TC], ones_sb[:], u2[:, oa:TC],
                                start=(i == 0), stop=False,
                            )
                        nc.tensor.matmul(
                            zp[:], ones_sb[0:4, :], wd4p_sb[k][:],
                            start=False, stop=True,
                        )
                        zb = sZb.tile([128, TC], FP32, tag="zb")
                        nc.vector.reciprocal_approx_fast(zb[:], zp[:])
                        return zb

                    def o_phase(k, us):
                        op = psO.tile([128, TC], FP32, tag="o")
                        for jb, off, uu in us:
                            nc.tensor.matmul(
                                op[:, off:TC], v_sb[:, jb, :], uu[:, off:TC],
                                start=(jb == 0), stop=(jb == len(us) - 1),
                            )
                        return op

                    def epilogue(k, op, zb):
                        c, h = units[k]
                        osum = sOs.tile([128, TC], FP32, tag="osum")
                        nc.vector.tensor_add(osum[:], op[:], tacc_sb[:, k, :])
                        nc.vector.tensor_mul(
                            oT_sb[:, h, _ts(c, TC)], osum[:], zb[:]
                        )

                    ndrain = [0]
                    res_map = {}

                    def outproj_half(tb, half):
                        # 2 nch, each: 2 head matmuls into a [128,512] psum
                        # single, drain alternating DVE/Act, DMA per block.
                        if half == 0:
                            res_map[tb] = sRes.tile([128, DM], BF16, tag="res",
                                                    name=f"res{tb}")
                        res = res_map[tb]
                        for s_ in range(2):
                            nch = half * 2 + s_
                            opp = psOut.tile([128, TC], FP32, tag="op",
                                             name=f"op{tb}_{nch}")
                            for h in range(NQH):
                                nc.tensor.matmul(
                                    opp[:], oT_sb[:, h, _ts(tb, 128)],
                                    wo_sb[:, h, _ts(nch, TC)],
                                    start=(h == 0), stop=(h == NQH - 1),
                                )
                            ndrain[0] += 1
                            dst = res[:, _ts(nch, TC)]
                            if ndrain[0] % 2 == 0:
                                nc.scalar.copy(dst, opp[:])
                            else:
                                nc.vector.tensor_copy(dst, opp[:])
                        if half == 1:
                            nc.sync.dma_start(out[_ts(tb, 128), :], res[:])
                            del res_map[tb]

                    def outproj_tail(tb, half, psum_src, split=False):
                        # tail variant: per-512 singles cycling the by-now
                        # idle psS/psO/psZ banks so the drain pipeline is
                        # wider than the lone psOut pair.
                        if half == 0:
                            res_map[tb] = sRes.tile([128, DM], BF16, tag="res", name=f"res{tb}")
                        res = res_map[tb]
                        for s_ in range(2):
                            nch = half * 2 + s_
                            pool, tag = psum_src[(2 * tb + half + s_) % len(psum_src)]
                            opp = pool.tile([128, TC], FP32, tag=tag,
                                            name=f"ot{tb}_{nch}")
                            for h in range(NQH):
                                nc.tensor.matmul(
                                    opp[:], oT_sb[:, h, _ts(tb, 128)],
                                    wo_sb[:, h, _ts(nch, TC)],
                                    start=(h == 0), stop=(h == NQH - 1),
                                )
                            ndrain[0] += 1
                            dst = res[:, _ts(nch, TC)]
                            if ndrain[0] % 2 == 0:
                                nc.scalar.copy(dst, opp[:])
                            else:
                                nc.vector.tensor_copy(dst, opp[:])
                        if split:
                            nc.sync.dma_start(
                                out[_ts(tb, 128),
                                    half * (DM // 2):(half + 1) * (DM // 2)],
                                res[:, half * (DM // 2):(half + 1) * (DM // 2)],
                            )
                            if half == 1:
                                del res_map[tb]
                        elif half == 1:
                            nc.sync.dma_start(out[_ts(tb, 128), :], res[:])
                            del res_map[tb]

                    pend_out = []

                    def drain_out(n):
                        while n > 0 and pend_out:
                            outproj_half(*pend_out.pop(0))
                            n -= 1

                    # --- interleaved emission: hoisted S units fill the PE
                    # while ropes/eu drain on DVE; depth fronts stay ahead
                    # of their z-phase use; taccs lag into phase B ---
                    saved = {}
                    fronts = {}
                    saved[0] = (s_phase(0),)
                    fronts[0] = depth_front(0)
                    saved[1] = (s_phase(1),)
                    fronts[1] = depth_front(1)
                    saved[2] = (s_phase(2),)
                    fronts[2] = depth_front(2)

                    for k in range(NU):
                        if k not in saved:
                            saved[k] = (s_phase(k),)
                        nf = k + 3
                        if nf < NU and nf not in fronts:
                            fronts[nf] = depth_front(nf)
                        if k in fronts:
                            depth_tacc(k, fronts.pop(k))
                        drain_out(2)
                        if k > 0:
                            us_prev, zparts_prev = saved[k - 1][0]
                            zb = z_phase(k - 1, zparts_prev)
                            drain_out(2)
                            op = o_phase(k - 1, us_prev)
                            saved[k - 1] = (op, zb)
                        if k > 1:
                            op, zb = saved.pop(k - 2)
                            epilogue(k - 2, op, zb)
                            c2, h2 = units[k - 2]
                            if h2 == NQH - 1:
                                pend_out.extend(
                                    (tb, half)
                                    for tb in range(c2 * 4, (c2 + 1) * 4)
                                    for half in range(2)
                                )
                            if k >= 6:
                                drain_out(2)

                    us7, zparts7 = saved[7][0]
                    zb7 = z_phase(7, zparts7)
                    drain_out(2)
                    op7 = o_phase(7, us7)
                    op6, zb6 = saved.pop(6)
                    epilogue(6, op6, zb6)
                    drain_out(2)
                    epilogue(7, op7, zb7)
                    c7, _ = units[7]
                    pend_out.extend(
                        (tb, half)
                        for tb in range(c7 * 4, (c7 + 1) * 4)
                        for half in range(2)
                    )
                    # tail: cycle psS/psO/psZ banks (idle by now) alongside
                    # psOut so drains pipeline across many banks
                    psum_src = [(psS, "s"), (psO, "o"), (psZ, "z"), (psOut, "op")]
                    while pend_out:
                        tb, half = pend_out.pop(0)
                        outproj_tail(tb, half, psum_src,
                                     split=len(pend_out) < 4)

    nc.compile()
    return nc


def get_program(T):
    if T not in _programs:
        _programs[T] = build_program(T)
    return _programs[T]


def make_in_maps(x, depth_k, depth_v, cos, sin, Wq, Wk, Wv, Wo, T):
    xT16 = np.ascontiguousarray(x[0].T).astype(NPBF16)
    cosT16 = np.ascontiguousarray(cos[0, 0].T).astype(NPBF16)
    sinT16 = np.ascontiguousarray(sin[0, 0].T).astype(NPBF16)
    mask16 = np.triu(np.ones((128, 128), np.float32)).astype(NPBF16)
    in_maps = []
    for c in range(N_CORES):
        wq_c = np.ascontiguousarray(
            np.stack(
                [
                    Wq[:, (2 * c + h) * HD: (2 * c + h + 1) * HD]
                    .reshape(DK, 128, HD).transpose(1, 0, 2)
                    for h in range(NQH)
                ],
                axis=1,
            )
        ).astype(NPBF16)
        wk_c = np.ascontiguousarray(
            Wk[:, c * HD: (c + 1) * HD].reshape(DK, 128, HD).transpose(1, 0, 2)
        ).astype(NPBF16)
        wv_c = np.ascontiguousarray(
            Wv[:, c * HD: (c + 1) * HD].reshape(DK, 128, HD).transpose(1, 0, 2)
        ).astype(NPBF16)
        wo_c = np.ascontiguousarray(
            Wo[2 * c * HD: (2 * c + 2) * HD, :].reshape(NQH, HD, DM)
            .transpose(1, 0, 2)
        ).astype(NPBF16)
        kdT_c = np.ascontiguousarray(depth_k[:, 0, c].transpose(0, 2, 1)).astype(NPBF16)
        vdT_c = np.ascontiguousarray(depth_v[:, 0, c].transpose(0, 2, 1)).astype(NPBF16)
        in_maps.append(
            {
                "xT": xT16, "wq": wq_c, "wk": wk_c, "wv": wv_c, "wo": wo_c,
                "cosT": cosT16, "sinT": sinT16, "kdT": kdT_c, "vdT": vdT_c,
                "mask": mask16,
            }
        )
    return in_maps


def kernel(x, depth_k, depth_v, cos, sin, Wq, Wk, Wv, Wo):
    x = np.asarray(x, np.float32)
    T = x.shape[1]
    nc = get_program(T)
    in_maps = make_in_maps(
        x, np.asarray(depth_k, np.float32), np.asarray(depth_v, np.float32),
        np.asarray(cos, np.float32), np.asarray(sin, np.float32),
        np.asarray(Wq, np.float32), np.asarray(Wk, np.float32),
        np.asarray(Wv, np.float32), np.asarray(Wo, np.float32), T,
    )
    trace = bool(os.environ.get("MODA_TRACE"))
    res = run_bass_kernel_spmd(nc, in_maps, list(range(N_CORES)), trace=trace)
    global last_result
    last_result = res
    total = np.zeros((T, DM), np.float32)
    for c in range(N_CORES):
        total += res.results[c]["out"].astype(np.float32)
    return total.reshape(1, T, DM)
